# revision 13
# baseline (speedup 1.0000x reference)
# Trainium2 Bass kernel for the CTAM SSCL loss (camera-masked supervised
# contrastive loss over a memory bank + hard-positive gather).
#
# Distribution: the bank dimension N=65536 is sharded across 8 cores
# (8192 entries each). On each core the shard is laid out as
# [128 partitions, 4096 free] where partition p < 64 holds sample p's
# logits for the first half of the shard and partition p+64 the second
# half. All per-sample reductions (masked sum-exp, positive sum/count,
# masked min + argmin) are free-dim reductions, combined on the host in
# f64 across (core, half, chunk) partials. The hard-positive row is
# gathered from `mem` on the host after the global (value, index) argmin
# resolve. The masked max of the reference cancels analytically:
#   loss_i = log(sum_cam exp(l/T)) - (sum_pos l) / (T * npos).
#
# Mask arithmetic on device: key_j = CID_j*1024 + TID_j (exact in f32),
# Q = (key_j - key_anchor_i)^2. Then Q == 0 <=> positive (same cam and
# same track), Q <= 511^2 <=> same camera (the 1024 spacing makes the
# nearest other-camera key at least 513 away).

import numpy as np

B, N, D = 64, 65536, 2048
NCORES = 8
SHARD = N // NCORES  # 8192
HALF = SHARD // 2  # 4096 (free dim per partition row)
CHUNK = 2048
NCHUNKS = HALF // CHUNK  # 2
TEMP = 0.07
INV_TEMP = 1.0 / TEMP
KEYMUL = 1024.0
CAM_THR = 261800.0  # between 511^2=261121 (same cam) and 512^2=262144
POS_THR = 0.5
SBIG = 2000.0  # penalty scale for the masked-min path
NSTAT = 5  # per-chunk stats: Z, PSUM, NPOS, NEGMIN, IDX
NROWS = NSTAT * NCHUNKS

_CACHE = {}


def _build_nc(finalize=True):
    import concourse.bass as bass
    import concourse.bacc as bacc
    import concourse.tile as tile
    from concourse import mybir

    f32 = mybir.dt.float32
    Alu = mybir.AluOpType
    Act = mybir.ActivationFunctionType

    # Bacc (not raw Bass): its finalize() runs the nop->event-semaphore
    # legalization without which the multi-wait kernel-tail drain
    # overflows the ISA sync-wait slots at walrus codegen.
    nc = bacc.Bacc()
    lg = nc.dram_tensor("lg", [128, HALF], f32, kind="ExternalInput").ap()
    keys = nc.dram_tensor("keys", [2, HALF], f32, kind="ExternalInput").ap()
    negak = nc.dram_tensor("negak", [128, 1], f32, kind="ExternalInput").ap()
    stats = nc.dram_tensor("stats", [128, NROWS], f32, kind="ExternalOutput").ap()

    with (
        tile.TileContext(nc) as tc,
        tc.tile_pool(name="const", bufs=1) as constp,
        tc.tile_pool(name="qpool", bufs=2) as qpool,
        tc.tile_pool(name="epool", bufs=2) as epool,
        tc.tile_pool(name="spool", bufs=2) as spool,
        tc.tile_pool(name="tpool", bufs=2) as tpool,
        tc.tile_pool(name="mpool", bufs=2) as mpool,
        tc.tile_pool(name="scr", bufs=3) as scrp,
    ):
        negak_sb = constp.tile([128, 1], f32)
        nc.sync.dma_start(out=negak_sb[:], in_=negak[:])

        # Single full-width loads (keeps the DMA-queue count low: the
        # kernel-tail drain has a hard cap on distinct semaphore waits).
        l_full = constp.tile([128, HALF], f32)
        nc.sync.dma_start(out=l_full[:], in_=lg[:])

        # Broadcast keys rows (2 x HALF) to all 128 partitions with a
        # 0-stride DMA: partition p reads keys[p // 64, f].
        kb_full = constp.tile([128, HALF], f32)
        kb_bcast = bass.AP(
            tensor=keys.tensor,
            offset=keys.offset,
            ap=[[HALF, 2], [0, 64], [1, HALF]],
        )
        nc.gpsimd.dma_start(out=kb_full[:], in_=kb_bcast)

        # All 10 per-chunk accumulators land in columns of one staging
        # tile (sub-tile dep tracking keeps the writers independent);
        # a single DMA ships it out at the end.
        stats_sb = constp.tile([128, NROWS], f32)

        for c in range(NCHUNKS):
            off = c * CHUNK
            l_sb = l_full[:, off : off + CHUNK]

            def acc(r):
                col = c * NSTAT + r
                return stats_sb[:, col : col + 1]

            # Q = (key - anchor_key)^2 ; exact 0 at positives.
            q_sb = qpool.tile([128, CHUNK], f32)
            nc.scalar.activation(
                out=q_sb[:],
                in_=kb_full[:, off : off + CHUNK],
                func=Act.Square,
                bias=negak_sb[:, 0:1],
                scale=1.0,
            )

            # E = exp(l / TEMP); range-safe in f32 (|l| <~ 1.2 -> exp < 4e7).
            e_sb = epool.tile([128, CHUNK], f32)
            nc.scalar.activation(out=e_sb[:], in_=l_sb, func=Act.Exp, scale=INV_TEMP)

            # S = SBIG * Q: 0 at positives, >= SBIG elsewhere.
            s_sb = spool.tile([128, CHUNK], f32)
            nc.scalar.mul(out=s_sb[:], in_=q_sb[:], mul=SBIG)

            # npos partial = sum exp(-100*Q): 1 at positives, ~0 elsewhere
            # (host rounds the combined sum to the nearest integer).
            npos_scr = scrp.tile([128, CHUNK], f32, tag="scr")
            nc.scalar.activation(
                out=npos_scr[:],
                in_=q_sb[:],
                func=Act.Exp,
                scale=-100.0,
                accum_out=acc(2),
            )

            # Z partial = sum over same-camera of E.
            z_scr = scrp.tile([128, CHUNK], f32, tag="scr")
            nc.vector.scalar_tensor_tensor(
                out=z_scr[:],
                in0=q_sb[:],
                scalar=CAM_THR,
                in1=e_sb[:],
                op0=Alu.is_le,
                op1=Alu.mult,
                accum_out=acc(0),
            )

            # psum partial = sum over positives of l (exact: mask is 1/0).
            pl_scr = scrp.tile([128, CHUNK], f32, tag="scr")
            nc.vector.scalar_tensor_tensor(
                out=pl_scr[:],
                in0=q_sb[:],
                scalar=POS_THR,
                in1=l_sb,
                op0=Alu.is_le,
                op1=Alu.mult,
                accum_out=acc(1),
            )

            # Hard positive: argmin over positives of l == argmax of
            # TNEG = -S - l (exact -l at positives). The top-8 max +
            # max_index pair gives the max value and its first index
            # (first-occurrence tie behavior matches jnp.argmin).
            tneg_sb = tpool.tile([128, CHUNK], f32)
            nc.vector.scalar_tensor_tensor(
                out=tneg_sb[:],
                in0=s_sb[:],
                scalar=-1.0,
                in1=l_sb,
                op0=Alu.mult,
                op1=Alu.subtract,
            )
            mx8 = mpool.tile([128, 8], f32, tag="mx8")
            nc.vector.max(mx8[:], tneg_sb[:])
            ix8 = mpool.tile([128, 8], mybir.dt.uint32, tag="ix8")
            nc.vector.max_index(ix8[:], mx8[:], tneg_sb[:])
            # stats col 3 = -min (negated on host), col 4 = index (as f32)
            nc.gpsimd.tensor_copy(out=acc(3), in_=mx8[:, 0:1])
            nc.vector.tensor_copy(out=acc(4), in_=ix8[:, 0:1])

        nc.sync.dma_start(out=stats[:], in_=stats_sb[:])

    if finalize:
        nc.finalize()
    return nc


def _get_nc():
    if "nc" not in _CACHE:
        _CACHE["nc"] = _build_nc()
    return _CACHE["nc"]


def _make_in_maps(logits, mem_CID, mem_TID, camids, trackids):
    key = mem_CID.astype(np.float32) * np.float32(KEYMUL) + mem_TID.astype(np.float32)
    ak = camids.astype(np.float32) * np.float32(KEYMUL) + trackids.astype(np.float32)
    negak = np.ascontiguousarray(-np.tile(ak, 2).reshape(128, 1)).astype(np.float32)
    in_maps = []
    for k in range(NCORES):
        sh = logits[:, k * SHARD : (k + 1) * SHARD]
        lg = np.ascontiguousarray(
            np.concatenate([sh[:, :HALF], sh[:, HALF:]], axis=0), dtype=np.float32
        )
        ks = np.ascontiguousarray(
            key[k * SHARD : (k + 1) * SHARD].reshape(2, HALF), dtype=np.float32
        )
        in_maps.append({"lg": lg, "keys": ks, "negak": negak})
    return in_maps


def _combine(results, mem):
    Z = np.zeros(B, np.float64)
    PS = np.zeros(B, np.float64)
    NP_ = np.zeros(B, np.float64)
    best_val = np.full(B, np.inf)
    best_idx = np.zeros(B, np.int64)
    for k in range(NCORES):
        st = results[k]["stats"].astype(np.float64)  # [128, NROWS]
        for c in range(NCHUNKS):
            z = st[:, c * NSTAT + 0]
            ps = st[:, c * NSTAT + 1]
            npos = st[:, c * NSTAT + 2]
            mv = -st[:, c * NSTAT + 3]
            ix = st[:, c * NSTAT + 4]
            for half in range(2):
                rows = slice(half * 64, half * 64 + 64)
                Z += z[rows]
                PS += ps[rows]
                NP_ += npos[rows]
                gidx = k * SHARD + half * HALF + c * CHUNK + ix[rows].astype(np.int64)
                v = mv[rows]
                better = (v < best_val) | ((v == best_val) & (gidx < best_idx))
                best_val = np.where(better, v, best_val)
                best_idx = np.where(better, gidx, best_idx)
    npos = np.rint(NP_)
    loss = np.mean(np.log(Z) - PS / (TEMP * npos))
    hard_pos = np.ascontiguousarray(mem[best_idx], dtype=np.float32)
    return np.float32(loss), hard_pos


def kernel(logits, mem, mem_CID, mem_TID, camids, trackids):
    from concourse.bass_utils import run_bass_kernel_spmd

    logits = np.asarray(logits, np.float32)
    mem = np.asarray(mem, np.float32)
    in_maps = _make_in_maps(
        logits,
        np.asarray(mem_CID),
        np.asarray(mem_TID),
        np.asarray(camids),
        np.asarray(trackids),
    )
    nc = _get_nc()
    res = run_bass_kernel_spmd(nc, in_maps, core_ids=list(range(NCORES)))
    return _combine(res.results, mem)
